# revision 24
# baseline (speedup 1.0000x reference)
"""Trainium2 distributed kernel for a linear-recurrence associative scan.

    h_t = g_t * h_{t-1} + x_t  along the sequence axis (N=8192)

Shapes: gates/inputs [B=4, N=8192, D=1024] f32.

Strategy: the scan is independent per (b, d) lane -> 4096 lanes of length
8192.  Shard lanes across the 8 NeuronCores (512 lanes each), laid out
lane-major so each SBUF partition holds one lane's contiguous sequence and
the hardware scan instruction (tensor_tensor_scan: state = g*state + x along
the free dim, one recurrence per partition) does the whole recurrence at
line rate.  No collectives needed.

All HBM traffic is fp16 (the scan's recurrent state stays fp32 inside the
engine regardless of operand dtype, so the only error is per-element
quantization, ~1e-3 max rel err vs the 2e-2 gate).

The kernel is DVE-bound: the scan only runs on the vector engine (walrus
rejects TensorTensorScan on GpSimd), ~34us for 4 lane-tiles.  Each DMA's
transfer occupies only its issuing engine's queue, so the ~76us of HBM
traffic spreads over the three DMA-capable queues (SP, Activation,
Pool/GpSimd) at ~25us each, fully hidden under the scan chain.  Tiles
stream in chunks (small leading chunks so the first scan starts early,
small trailing chunks so the last output drains immediately).

Sync legality (walrus encodes at most ONE sem wait per instruction):
 * Every SBUF buffer is unique -- no slot-reuse WAR/WAW hazards.
 * Chained scan chunks would need two waits (input DMA + carry cell); a
   1-element copy absorber carries the DMA wait first, so the scan keeps
   only the carry wait.
 * HWDGE DMA completions share 8 global counting sems assigned round-robin
   in tick order; the 9th+ DMA carries a lane-reuse (WAW) wait.  Output
   DMAs also need a scan wait, so their lane wait must be elided: each
   HWDGE ring (SP / Act) chains its input DMAs with sync deps (~70ns each;
   the ring's engine clock then observes its own lanes) and issues at most
   3 trailing output DMAs, pinned (nosync deps) into a strict alternating
   global tick order so every out's lane predecessor is a chain-observed
   same-ring input.  Outputs spanning several scan chunks still carry ONE
   wait: all DVE ops tick one counting sem, so the join is a single value.
 * Pool's SWDGE DMAs use a separate 8-sem pool and stay under 8 total, so
   they never recycle and carry only their data wait.
 * Tile's kernel-tail drain would wait on every live sem at once; a custom
   drain splits it into a ladder of single-wait NOPs.
"""

import numpy as np

B, N, D = 4, 8192, 1024
N_CORES = 8
LANES = B * D  # 4096 independent recurrences
LANES_PER_CORE = LANES // N_CORES  # 512
P = 128  # SBUF partitions
LANE_TILES = LANES_PER_CORE // P  # 4

# per-tile scan-chunk boundaries (columns)
CHUNKS = {
    0: [0, 1024, 4096, 8192],
    1: [0, 2048, 5120, 8192],
    2: [0, 2048, 5120, 8192],
    3: [0, 3072, 6144, 7168, 7680, 8192],
}
# per-tile input-DMA boundaries; every scan chunk must lie inside a single
# input DMA (so the scan carries at most one data wait)
IN_CHUNKS = {
    0: [0, 1024, 4096, 8192],
    1: [0, 2048, 5120, 8192],
    2: [0, 2048, 5120, 8192],
    3: [0, 3072, 6144, 8192],
}
# input DMAs: (engine, tile, chunk) in emission order (= per-ring runtime
# order).  SP also carries tile 2's last chunk so Pool stays within the
# 8-sem SWDGE budget after taking two tile-3 output DMAs.
INS = [
    ("sp", 0, 0), ("act", 1, 0), ("pool", 2, 0),
    ("sp", 0, 1), ("act", 1, 1), ("pool", 2, 1),
    ("sp", 0, 2), ("act", 1, 2), ("pool", 3, 0),
    ("sp", 2, 2), ("pool", 3, 1), ("pool", 3, 2),
]
# DVE scan order: interleave tiles 0/1/2 by arrival, then tile 3
SCAN_ORDER = [
    (0, 0), (1, 0), (2, 0),
    (0, 1), (1, 1), (2, 1),
    (0, 2), (1, 2), (2, 2),
    (3, 0), (3, 1), (3, 2), (3, 3), (3, 4),
]
# outputs: (engine, tile, lo, hi); ring outs in readiness order, at most 3
# per HWDGE ring; tile 3's early chunks drain on the otherwise-idle Pool
OUTS = [
    ("sp", 0, 0, 8192),
    ("act", 1, 0, 8192),
    ("pool", 3, 0, 3072),
    ("sp", 2, 0, 8192),
    ("pool", 3, 3072, 6144),
    ("sp", 3, 6144, 7168),
    ("act", 3, 7168, 7680),
    ("act", 3, 7680, 8192),
]
# HWDGE compile-time tick order (fixes the 8-lane round-robin): ins first,
# arranged so each ring out's lane predecessor (8 ticks back) is an input
# on the SAME ring whose completion the ring's in-chain already observed.
# Labels refer to INS/OUTS entries.
PIN_ORDER = [
    ("in", "sp", 0, 0),   # L0
    ("in", "act", 1, 0),  # L1
    ("in", "sp", 0, 1),   # L2
    ("in", "sp", 0, 2),   # L3
    ("in", "act", 1, 1),  # L4
    ("in", "act", 1, 2),  # L5
    ("in", "sp", 2, 2),   # L6
    ("out", "act", 1, 0, 8192),     # L7 fresh
    ("out", "sp", 0, 0, 8192),      # L0 <- sp in(0,0), chain-observed
    ("out", "act", 3, 7168, 7680),  # L1 <- act in(1,0), chain-observed
    ("out", "sp", 2, 0, 8192),      # L2 <- sp in(0,1), chain-observed
    ("out", "sp", 3, 6144, 7168),   # L3 <- sp in(0,2), chain-observed
    ("out", "act", 3, 7680, 8192),  # L4 <- act in(1,1), chain-observed
]

# Conservative fallback schedule (verified legal and correct on HW at
# ~78us): whole-tile chunks, everything on the SP queue -- exactly 8 HWDGE
# DMAs, no lane recycling, no pins needed.  Used only if the aggressive
# schedule ever fails the build-time one-wait audit (e.g. a Tile scheduler
# change shifts the lane rotation).
FB_CHUNKS = {t: [0, N] for t in range(LANE_TILES)}
FB_INS = [("sp", t, 0) for t in range(LANE_TILES)]
FB_SCAN_ORDER = [(t, 0) for t in range(LANE_TILES)]
FB_OUTS = [("sp", t, 0, N) for t in range(LANE_TILES)]

_NC_CACHE = None


def _build_bass(fallback=False):
    import concourse.bass as bass
    import concourse.tile as tile
    from concourse import mybir
    from concourse.vector_clock import ScopedClock, VectorClock

    class OneWaitDrainTC(tile.TileContext):
        """Split the kernel-tail drain's multi-sem wait into a ladder of
        single-wait NOPs (walrus allows one sync-wait per instruction)."""

        def _drain_and_barrier(self, tick_clock, wait_clock):
            full = tick_clock.global_clock
            n = len(full)
            for proc in range(n):
                t = full[proc]
                if t <= 0:
                    continue
                partial = VectorClock([0] * n)
                partial.require_at_least(proc, t)
                nop = self.nc.sync.nop(hint=f"drainwait{proc}")
                wait_clock.add_sem_waits(nop.ins, ScopedClock({None: partial}))
            self.nc.sync.drain()
            self.nc.all_engine_barrier()
            assert self.sems is not None
            popped = self.nc._tile_sem_poison_stack.pop()
            assert popped is self._sem_poison
            self.nc.clear_and_free_semaphores(list(self.sems.allocated().values()))
            self.nc.all_engine_barrier()

    chunks = FB_CHUNKS if fallback else CHUNKS
    in_chunks = FB_CHUNKS if fallback else IN_CHUNKS
    ins_tab = FB_INS if fallback else INS
    scan_tab = FB_SCAN_ORDER if fallback else SCAN_ORDER
    outs_tab = FB_OUTS if fallback else OUTS
    pin_tab = [] if fallback else PIN_ORDER

    f16 = mybir.dt.float16
    nc = bass.Bass()
    gx_ext = nc.declare_dram_parameter(
        "gx", [LANES_PER_CORE, 2 * N], f16, isOutput=False
    )
    o_ext = nc.declare_dram_parameter("out", [LANES_PER_CORE, N], f16, isOutput=True)

    with OneWaitDrainTC(nc) as tc:
        with tc.tile_pool(name="p", bufs=1) as tp:
            gxt = [tp.tile([P, 2, N], f16, name=f"gx{t}") for t in range(LANE_TILES)]
            ot = [tp.tile([P, N], f16, name=f"o{t}") for t in range(LANE_TILES)]
            scr = tp.tile([P, 16], f16, name="scr")

            def gxd(t):
                return gx_ext[t * P : (t + 1) * P, :].rearrange(
                    "p (a n) -> p a n", n=N
                )

            engs = {"sp": nc.sync, "act": nc.scalar, "pool": nc.gpsimd}
            ring_last = {"sp": None, "act": None}
            in_dmas = {}  # (eng, tile, chunk) -> instruction

            def din(e, t, c):
                lo, hi = in_chunks[t][c], in_chunks[t][c + 1]
                d = engs[e].dma_start(
                    out=gxt[t][:, :, lo:hi], in_=gxd(t)[:, :, lo:hi]
                )
                if e in ring_last:
                    if ring_last[e] is not None:
                        tile.add_dep_helper(
                            d.ins, ring_last[e].ins, sync=True, reason="in chain"
                        )
                    ring_last[e] = d
                in_dmas[(e, t, c)] = d
                return d

            for e, t, c in ins_tab:
                din(e, t, c)

            nabs = [0]

            def scan(t, c):
                lo, hi = chunks[t][c], chunks[t][c + 1]
                if c > 0:
                    # absorber carries the input-DMA wait; the scan keeps
                    # only its carry-cell wait
                    nc.vector.tensor_copy(
                        scr[:, nabs[0] : nabs[0] + 1], gxt[t][:, 0, lo : lo + 1]
                    )
                    nabs[0] += 1
                init = 0.0 if c == 0 else ot[t][:, lo - 1 : lo]
                nc.vector.tensor_tensor_scan(
                    ot[t][:, lo:hi],
                    gxt[t][:, 0, lo:hi],
                    gxt[t][:, 1, lo:hi],
                    init,
                    mybir.AluOpType.mult,
                    mybir.AluOpType.add,
                )

            for t, c in scan_tab:
                scan(t, c)

            out_dmas = {}
            for e, t, lo, hi in outs_tab:
                d = engs[e].dma_start(
                    out=o_ext[t * P : (t + 1) * P, lo:hi], in_=ot[t][:, lo:hi]
                )
                out_dmas[(e, t, lo, hi)] = d

            # nosync pin chain: fixes HWDGE tick order (hence lane
            # round-robin) at compile time without any runtime waits
            prev = None
            for entry in pin_tab:
                if entry[0] == "in":
                    d = in_dmas[entry[1:]]
                else:
                    d = out_dmas[entry[1:]]
                if prev is not None:
                    tile.add_dep_helper(d.ins, prev.ins, sync=False, reason="pin")
                prev = d

    # one sync-wait per instruction is a hard walrus limit -- catch
    # regressions at build time rather than at NEFF compile
    for name, inst in nc.inst_map.items():
        si = inst.sync_info
        nw = len(si.on_wait) if si and si.on_wait else 0
        assert nw <= 1, f"{name} ({inst.engine}) carries {nw} sem waits"
    return nc


def _get_nc():
    global _NC_CACHE
    if _NC_CACHE is None:
        try:
            _NC_CACHE = _build_bass()
        except AssertionError:
            # one-wait audit failed -- fall back to the conservative
            # single-queue schedule rather than not running at all
            _NC_CACHE = _build_bass(fallback=True)
    return _NC_CACHE


def kernel(gates: np.ndarray, inputs: np.ndarray) -> np.ndarray:
    import os

    # The axon client here has no NTFF profile hook (antenv.axon_hooks);
    # make sure run_bass_kernel_spmd never takes the trace path even if
    # BASS_TRACE is set in the environment.
    os.environ["BASS_NEVER_TRACE"] = "1"
    from concourse.bass_utils import run_bass_kernel_spmd

    # [B, N, D] -> lane-major [B*D, N] fp16; pack gates|inputs along columns
    gt = np.asarray(gates, dtype=np.float32).transpose(0, 2, 1).reshape(LANES, N)
    xt = np.asarray(inputs, dtype=np.float32).transpose(0, 2, 1).reshape(LANES, N)
    gx = np.empty((LANES, 2 * N), dtype=np.float16)
    gx[:, :N] = gt
    gx[:, N:] = xt

    in_maps = [
        {"gx": gx[i * LANES_PER_CORE : (i + 1) * LANES_PER_CORE]}
        for i in range(N_CORES)
    ]
    try:
        res = run_bass_kernel_spmd(_get_nc(), in_maps, core_ids=list(range(N_CORES)))
    except Exception:
        # One retry: the device recovers from transient NRT execution
        # faults, and the NEFF is cached so the retry is cheap.
        res = run_bass_kernel_spmd(_get_nc(), in_maps, core_ids=list(range(N_CORES)))
    out = np.concatenate([res.results[i]["out"] for i in range(N_CORES)], axis=0)
    # [B*D, N] fp16 -> [B, N, D] f32
    return np.ascontiguousarray(
        out.astype(np.float32).reshape(B, D, N).transpose(0, 2, 1)
    )


# revision 25
# speedup vs baseline: 1.0073x; 1.0073x over previous
"""Trainium2 distributed kernel for a linear-recurrence associative scan.

    h_t = g_t * h_{t-1} + x_t  along the sequence axis (N=8192)

Shapes: gates/inputs [B=4, N=8192, D=1024] f32.

Strategy: the scan is independent per (b, d) lane -> 4096 lanes of length
8192.  Shard lanes across the 8 NeuronCores (512 lanes each), laid out
lane-major so each SBUF partition holds one lane's contiguous sequence and
the hardware scan instruction (tensor_tensor_scan: state = g*state + x along
the free dim, one recurrence per partition) does the whole recurrence at
line rate.  No collectives needed.

All HBM traffic is fp16 (the scan's recurrent state stays fp32 inside the
engine regardless of operand dtype, so the only error is per-element
quantization, ~1e-3 max rel err vs the 2e-2 gate).

The kernel is DVE-bound: the scan only runs on the vector engine (walrus
rejects TensorTensorScan on GpSimd), ~34us for 4 lane-tiles.  Each DMA's
transfer occupies only its issuing engine's queue, so the ~76us of HBM
traffic spreads over the three DMA-capable queues (SP, Activation,
Pool/GpSimd) at ~25us each, fully hidden under the scan chain.  Tiles
stream in chunks (small leading chunks so the first scan starts early,
small trailing chunks so the last output drains immediately).

Sync legality (walrus encodes at most ONE sem wait per instruction):
 * Every SBUF buffer is unique -- no slot-reuse WAR/WAW hazards.
 * Chained scan chunks would need two waits (input DMA + carry cell); a
   1-element copy absorber carries the DMA wait first, so the scan keeps
   only the carry wait.
 * HWDGE DMA completions share 8 global counting sems assigned round-robin
   in tick order; the 9th+ DMA carries a lane-reuse (WAW) wait.  Output
   DMAs also need a scan wait, so their lane wait must be elided: each
   HWDGE ring (SP / Act) chains its input DMAs with sync deps (~70ns each;
   the ring's engine clock then observes its own lanes) and issues at most
   3 trailing output DMAs, pinned (nosync deps) into a strict alternating
   global tick order so every out's lane predecessor is a chain-observed
   same-ring input.  Outputs spanning several scan chunks still carry ONE
   wait: all DVE ops tick one counting sem, so the join is a single value.
 * Pool's SWDGE DMAs use a separate 8-sem pool and stay under 8 total, so
   they never recycle and carry only their data wait.
 * Tile's kernel-tail drain would wait on every live sem at once; a custom
   drain splits it into a ladder of single-wait NOPs.
"""

import numpy as np

B, N, D = 4, 8192, 1024
N_CORES = 8
LANES = B * D  # 4096 independent recurrences
LANES_PER_CORE = LANES // N_CORES  # 512
P = 128  # SBUF partitions
LANE_TILES = LANES_PER_CORE // P  # 4

# per-tile scan-chunk boundaries (columns)
CHUNKS = {
    0: [0, 1024, 4096, 8192],
    1: [0, 2048, 5120, 8192],
    2: [0, 2048, 5120, 8192],
    3: [0, 3072, 6144, 7168, 7680, 8192],
}
# per-tile input-DMA boundaries; every scan chunk must lie inside a single
# input DMA (so the scan carries at most one data wait)
IN_CHUNKS = {
    0: [0, 1024, 4096, 8192],
    1: [0, 2048, 5120, 8192],
    2: [0, 2048, 5120, 8192],
    3: [0, 3072, 6144, 8192],
}
# input DMAs: (engine, tile, chunk) in emission order (= per-ring runtime
# order).  SP also carries tile 2's last chunk so Pool stays within the
# 8-sem SWDGE budget after taking two tile-3 output DMAs.
INS = [
    ("sp", 0, 0), ("act", 1, 0), ("pool", 2, 0),
    ("sp", 0, 1), ("act", 1, 1), ("pool", 2, 1),
    ("sp", 0, 2), ("act", 1, 2), ("pool", 3, 0),
    ("sp", 2, 2), ("pool", 3, 1), ("pool", 3, 2),
]
# DVE scan order: interleave tiles 0/1/2 by arrival, then tile 3
SCAN_ORDER = [
    (0, 0), (1, 0), (2, 0),
    (0, 1), (1, 1), (2, 1),
    (0, 2), (1, 2), (2, 2),
    (3, 0), (3, 1), (3, 2), (3, 3), (3, 4),
]
# outputs: (engine, tile, lo, hi); ring outs in readiness order, at most 3
# per HWDGE ring; tile 3's early chunks drain on the otherwise-idle Pool
OUTS = [
    ("sp", 0, 0, 8192),
    ("act", 1, 0, 8192),
    ("pool", 3, 0, 3072),
    ("sp", 2, 0, 8192),
    ("pool", 3, 3072, 6144),
    ("sp", 3, 6144, 7168),
    ("act", 3, 7168, 7680),
    ("act", 3, 7680, 8192),
]
# HWDGE compile-time tick order (fixes the 8-lane round-robin): ins first,
# arranged so each ring out's lane predecessor (8 ticks back) is an input
# on the SAME ring whose completion the ring's in-chain already observed.
# Labels refer to INS/OUTS entries.
PIN_ORDER = [
    ("in", "sp", 0, 0),   # L0
    ("in", "act", 1, 0),  # L1
    ("in", "sp", 0, 1),   # L2
    ("in", "sp", 0, 2),   # L3
    ("in", "act", 1, 1),  # L4
    ("in", "act", 1, 2),  # L5
    ("in", "sp", 2, 2),   # L6
    ("out", "act", 1, 0, 8192),     # L7 fresh
    ("out", "sp", 0, 0, 8192),      # L0 <- sp in(0,0), chain-observed
    ("out", "act", 3, 7168, 7680),  # L1 <- act in(1,0), chain-observed
    ("out", "sp", 2, 0, 8192),      # L2 <- sp in(0,1), chain-observed
    ("out", "sp", 3, 6144, 7168),   # L3 <- sp in(0,2), chain-observed
    ("out", "act", 3, 7680, 8192),  # L4 <- act in(1,1), chain-observed
]

# Conservative fallback schedule (verified legal and correct on HW at
# ~78us): whole-tile chunks, everything on the SP queue -- exactly 8 HWDGE
# DMAs, no lane recycling, no pins needed.  Used only if the aggressive
# schedule ever fails the build-time one-wait audit (e.g. a Tile scheduler
# change shifts the lane rotation).
FB_CHUNKS = {t: [0, N] for t in range(LANE_TILES)}
FB_INS = [("sp", t, 0) for t in range(LANE_TILES)]
FB_SCAN_ORDER = [(t, 0) for t in range(LANE_TILES)]
FB_OUTS = [("sp", t, 0, N) for t in range(LANE_TILES)]

_NC_CACHE = None


def _build_bass(fallback=False):
    import concourse.bass as bass
    import concourse.tile as tile
    from concourse import mybir
    from concourse.vector_clock import ScopedClock, VectorClock

    class OneWaitDrainTC(tile.TileContext):
        """Split the kernel-tail drain's multi-sem wait into a ladder of
        single-wait NOPs (walrus allows one sync-wait per instruction)."""

        def _drain_and_barrier(self, tick_clock, wait_clock):
            full = tick_clock.global_clock
            n = len(full)
            for proc in range(n):
                t = full[proc]
                if t <= 0:
                    continue
                partial = VectorClock([0] * n)
                partial.require_at_least(proc, t)
                nop = self.nc.sync.nop(hint=f"drainwait{proc}")
                wait_clock.add_sem_waits(nop.ins, ScopedClock({None: partial}))
            self.nc.sync.drain()
            self.nc.all_engine_barrier()
            assert self.sems is not None
            popped = self.nc._tile_sem_poison_stack.pop()
            assert popped is self._sem_poison
            self.nc.clear_and_free_semaphores(list(self.sems.allocated().values()))

    chunks = FB_CHUNKS if fallback else CHUNKS
    in_chunks = FB_CHUNKS if fallback else IN_CHUNKS
    ins_tab = FB_INS if fallback else INS
    scan_tab = FB_SCAN_ORDER if fallback else SCAN_ORDER
    outs_tab = FB_OUTS if fallback else OUTS
    pin_tab = [] if fallback else PIN_ORDER

    f16 = mybir.dt.float16
    nc = bass.Bass()
    gx_ext = nc.declare_dram_parameter(
        "gx", [LANES_PER_CORE, 2 * N], f16, isOutput=False
    )
    o_ext = nc.declare_dram_parameter("out", [LANES_PER_CORE, N], f16, isOutput=True)

    with OneWaitDrainTC(nc) as tc:
        with tc.tile_pool(name="p", bufs=1) as tp:
            gxt = [tp.tile([P, 2, N], f16, name=f"gx{t}") for t in range(LANE_TILES)]
            ot = [tp.tile([P, N], f16, name=f"o{t}") for t in range(LANE_TILES)]
            scr = tp.tile([P, 16], f16, name="scr")

            def gxd(t):
                return gx_ext[t * P : (t + 1) * P, :].rearrange(
                    "p (a n) -> p a n", n=N
                )

            engs = {"sp": nc.sync, "act": nc.scalar, "pool": nc.gpsimd}
            ring_last = {"sp": None, "act": None}
            in_dmas = {}  # (eng, tile, chunk) -> instruction

            def din(e, t, c):
                lo, hi = in_chunks[t][c], in_chunks[t][c + 1]
                d = engs[e].dma_start(
                    out=gxt[t][:, :, lo:hi], in_=gxd(t)[:, :, lo:hi]
                )
                if e in ring_last:
                    if ring_last[e] is not None:
                        tile.add_dep_helper(
                            d.ins, ring_last[e].ins, sync=True, reason="in chain"
                        )
                    ring_last[e] = d
                in_dmas[(e, t, c)] = d
                return d

            for e, t, c in ins_tab:
                din(e, t, c)

            nabs = [0]

            def scan(t, c):
                lo, hi = chunks[t][c], chunks[t][c + 1]
                if c > 0:
                    # absorber carries the input-DMA wait; the scan keeps
                    # only its carry-cell wait
                    nc.vector.tensor_copy(
                        scr[:, nabs[0] : nabs[0] + 1], gxt[t][:, 0, lo : lo + 1]
                    )
                    nabs[0] += 1
                init = 0.0 if c == 0 else ot[t][:, lo - 1 : lo]
                nc.vector.tensor_tensor_scan(
                    ot[t][:, lo:hi],
                    gxt[t][:, 0, lo:hi],
                    gxt[t][:, 1, lo:hi],
                    init,
                    mybir.AluOpType.mult,
                    mybir.AluOpType.add,
                )

            for t, c in scan_tab:
                scan(t, c)

            out_dmas = {}
            for e, t, lo, hi in outs_tab:
                d = engs[e].dma_start(
                    out=o_ext[t * P : (t + 1) * P, lo:hi], in_=ot[t][:, lo:hi]
                )
                out_dmas[(e, t, lo, hi)] = d

            # nosync pin chain: fixes HWDGE tick order (hence lane
            # round-robin) at compile time without any runtime waits
            prev = None
            for entry in pin_tab:
                if entry[0] == "in":
                    d = in_dmas[entry[1:]]
                else:
                    d = out_dmas[entry[1:]]
                if prev is not None:
                    tile.add_dep_helper(d.ins, prev.ins, sync=False, reason="pin")
                prev = d

    # one sync-wait per instruction is a hard walrus limit -- catch
    # regressions at build time rather than at NEFF compile
    for name, inst in nc.inst_map.items():
        si = inst.sync_info
        nw = len(si.on_wait) if si and si.on_wait else 0
        assert nw <= 1, f"{name} ({inst.engine}) carries {nw} sem waits"
    return nc


def _get_nc():
    global _NC_CACHE
    if _NC_CACHE is None:
        try:
            _NC_CACHE = _build_bass()
        except AssertionError:
            # one-wait audit failed -- fall back to the conservative
            # single-queue schedule rather than not running at all
            _NC_CACHE = _build_bass(fallback=True)
    return _NC_CACHE


def kernel(gates: np.ndarray, inputs: np.ndarray) -> np.ndarray:
    import os

    # The axon client here has no NTFF profile hook (antenv.axon_hooks);
    # make sure run_bass_kernel_spmd never takes the trace path even if
    # BASS_TRACE is set in the environment.
    os.environ["BASS_NEVER_TRACE"] = "1"
    from concourse.bass_utils import run_bass_kernel_spmd

    # [B, N, D] -> lane-major [B*D, N] fp16; pack gates|inputs along columns
    gt = np.asarray(gates, dtype=np.float32).transpose(0, 2, 1).reshape(LANES, N)
    xt = np.asarray(inputs, dtype=np.float32).transpose(0, 2, 1).reshape(LANES, N)
    gx = np.empty((LANES, 2 * N), dtype=np.float16)
    gx[:, :N] = gt
    gx[:, N:] = xt

    in_maps = [
        {"gx": gx[i * LANES_PER_CORE : (i + 1) * LANES_PER_CORE]}
        for i in range(N_CORES)
    ]
    try:
        res = run_bass_kernel_spmd(_get_nc(), in_maps, core_ids=list(range(N_CORES)))
    except Exception:
        # One retry: the device recovers from transient NRT execution
        # faults, and the NEFF is cached so the retry is cheap.
        res = run_bass_kernel_spmd(_get_nc(), in_maps, core_ids=list(range(N_CORES)))
    out = np.concatenate([res.results[i]["out"] for i in range(N_CORES)], axis=0)
    # [B*D, N] fp16 -> [B, N, D] f32
    return np.ascontiguousarray(
        out.astype(np.float32).reshape(B, D, N).transpose(0, 2, 1)
    )


# revision 26
# speedup vs baseline: 1.0093x; 1.0019x over previous
"""Trainium2 distributed kernel for a linear-recurrence associative scan.

    h_t = g_t * h_{t-1} + x_t  along the sequence axis (N=8192)

Shapes: gates/inputs [B=4, N=8192, D=1024] f32.

Strategy: the scan is independent per (b, d) lane -> 4096 lanes of length
8192.  Shard lanes across the 8 NeuronCores (512 lanes each), laid out
lane-major so each SBUF partition holds one lane's contiguous sequence and
the hardware scan instruction (tensor_tensor_scan: state = g*state + x along
the free dim, one recurrence per partition) does the whole recurrence at
line rate.  No collectives needed.

All HBM traffic is fp16 (the scan's recurrent state stays fp32 inside the
engine regardless of operand dtype, so the only error is per-element
quantization, ~1e-3 max rel err vs the 2e-2 gate).

The kernel is DVE-bound: the scan only runs on the vector engine (walrus
rejects TensorTensorScan on GpSimd), ~34us for 4 lane-tiles.  Each DMA's
transfer occupies only its issuing engine's queue, so the ~76us of HBM
traffic spreads over the three DMA-capable queues (SP, Activation,
Pool/GpSimd) at ~25us each, fully hidden under the scan chain.  Tiles
stream in chunks (small leading chunks so the first scan starts early,
small trailing chunks so the last output drains immediately).

Sync legality (walrus encodes at most ONE sem wait per instruction):
 * Every SBUF buffer is unique -- no slot-reuse WAR/WAW hazards.
 * Chained scan chunks would need two waits (input DMA + carry cell); a
   1-element copy absorber carries the DMA wait first, so the scan keeps
   only the carry wait.
 * HWDGE DMA completions share 8 global counting sems assigned round-robin
   in tick order; the 9th+ DMA carries a lane-reuse (WAW) wait.  Output
   DMAs also need a scan wait, so their lane wait must be elided: each
   HWDGE ring (SP / Act) chains its input DMAs with sync deps (~70ns each;
   the ring's engine clock then observes its own lanes) and issues at most
   3 trailing output DMAs, pinned (nosync deps) into a strict alternating
   global tick order so every out's lane predecessor is a chain-observed
   same-ring input.  Outputs spanning several scan chunks still carry ONE
   wait: all DVE ops tick one counting sem, so the join is a single value.
 * Pool's SWDGE DMAs use a separate 8-sem pool and stay under 8 total, so
   they never recycle and carry only their data wait.
 * Tile's kernel-tail drain would wait on every live sem at once; a custom
   drain splits it into a ladder of single-wait NOPs.
"""

import numpy as np

B, N, D = 4, 8192, 1024
N_CORES = 8
LANES = B * D  # 4096 independent recurrences
LANES_PER_CORE = LANES // N_CORES  # 512
P = 128  # SBUF partitions
LANE_TILES = LANES_PER_CORE // P  # 4

# per-tile scan-chunk boundaries (columns)
CHUNKS = {
    0: [0, 512, 1024, 4096, 8192],
    1: [0, 1024, 2048, 5120, 8192],
    2: [0, 2048, 5120, 8192],
    3: [0, 3072, 6144, 7168, 7680, 8192],
}
# per-tile input-DMA boundaries; every scan chunk must lie inside a single
# input DMA (so the scan carries at most one data wait)
IN_CHUNKS = {
    0: [0, 512, 1024, 4096, 8192],
    1: [0, 1024, 2048, 5120, 8192],
    2: [0, 2048, 5120, 8192],
    3: [0, 3072, 6144, 8192],
}
# input DMAs: (engine, tile, chunk) in emission order (= per-ring runtime
# order).  SP also carries tile 2's last chunk so Pool stays within the
# 8-sem SWDGE budget after taking two tile-3 output DMAs.
INS = [
    ("sp", 0, 0), ("act", 1, 0), ("pool", 2, 0),
    ("sp", 0, 1), ("act", 1, 1), ("pool", 2, 1),
    ("sp", 0, 2), ("act", 1, 2), ("pool", 3, 0),
    ("sp", 0, 3), ("act", 1, 3), ("pool", 3, 1),
    ("sp", 2, 2), ("pool", 3, 2),
]
# the 9th HWDGE input recycles lane L0; chaining it to its LANE PREDECESSOR
# makes the chain wait and the lane WAW wait one and the same sem wait
CHAIN_OVERRIDE = {("sp", 2, 2): ("sp", 0, 0)}
# DVE scan order: interleave tiles 0/1/2 by arrival, then tile 3
SCAN_ORDER = [
    (0, 0), (1, 0), (0, 1), (1, 1), (2, 0),
    (0, 2), (1, 2), (2, 1),
    (0, 3), (1, 3), (2, 2),
    (3, 0), (3, 1), (3, 2), (3, 3), (3, 4),
]
# outputs: (engine, tile, lo, hi); ring outs in readiness order, at most 3
# per HWDGE ring; tile 3's early chunks drain on the otherwise-idle Pool
OUTS = [
    ("sp", 0, 0, 8192),
    ("act", 1, 0, 8192),
    ("pool", 3, 0, 3072),
    ("sp", 2, 0, 8192),
    ("pool", 3, 3072, 6144),
    ("pool", 3, 6144, 7168),
    ("act", 3, 7168, 7680),
    ("act", 3, 7680, 8192),
]
# HWDGE compile-time tick order (fixes the 8-lane round-robin): ins first,
# arranged so each ring out's lane predecessor (8 ticks back) is an input
# on the SAME ring whose completion the ring's in-chain already observed.
# Labels refer to INS/OUTS entries.
PIN_ORDER = [
    ("in", "sp", 0, 0),   # L0
    ("in", "act", 1, 0),  # L1
    ("in", "sp", 0, 1),   # L2
    ("in", "act", 1, 1),  # L3
    ("in", "sp", 0, 2),   # L4
    ("in", "act", 1, 2),  # L5
    ("in", "sp", 0, 3),   # L6
    ("in", "act", 1, 3),  # L7
    ("in", "sp", 2, 2),   # L0 again; chain==lane wait via CHAIN_OVERRIDE
    ("out", "act", 1, 0, 8192),     # L1 <- act in(1,0), chain-observed
    ("out", "sp", 0, 0, 8192),      # L2 <- sp in(0,1), chain-observed
    ("out", "act", 3, 7168, 7680),  # L3 <- act in(1,1), chain-observed
    ("out", "sp", 2, 0, 8192),      # L4 <- sp in(0,2), chain-observed
    ("out", "act", 3, 7680, 8192),  # L5 <- act in(1,2), chain-observed
]

# Conservative fallback schedule (verified legal and correct on HW at
# ~78us): whole-tile chunks, everything on the SP queue -- exactly 8 HWDGE
# DMAs, no lane recycling, no pins needed.  Used only if the aggressive
# schedule ever fails the build-time one-wait audit (e.g. a Tile scheduler
# change shifts the lane rotation).
FB_CHUNKS = {t: [0, N] for t in range(LANE_TILES)}
FB_INS = [("sp", t, 0) for t in range(LANE_TILES)]
FB_SCAN_ORDER = [(t, 0) for t in range(LANE_TILES)]
FB_OUTS = [("sp", t, 0, N) for t in range(LANE_TILES)]

_NC_CACHE = None


def _build_bass(fallback=False):
    import concourse.bass as bass
    import concourse.tile as tile
    from concourse import mybir
    from concourse.vector_clock import ScopedClock, VectorClock

    class OneWaitDrainTC(tile.TileContext):
        """Split the kernel-tail drain's multi-sem wait into a ladder of
        single-wait NOPs (walrus allows one sync-wait per instruction)."""

        def _drain_and_barrier(self, tick_clock, wait_clock):
            full = tick_clock.global_clock
            n = len(full)
            for proc in range(n):
                t = full[proc]
                if t <= 0:
                    continue
                partial = VectorClock([0] * n)
                partial.require_at_least(proc, t)
                nop = self.nc.sync.nop(hint=f"drainwait{proc}")
                wait_clock.add_sem_waits(nop.ins, ScopedClock({None: partial}))
            self.nc.sync.drain()
            self.nc.all_engine_barrier()
            assert self.sems is not None
            popped = self.nc._tile_sem_poison_stack.pop()
            assert popped is self._sem_poison
            self.nc.clear_and_free_semaphores(list(self.sems.allocated().values()))

    chunks = FB_CHUNKS if fallback else CHUNKS
    in_chunks = FB_CHUNKS if fallback else IN_CHUNKS
    ins_tab = FB_INS if fallback else INS
    scan_tab = FB_SCAN_ORDER if fallback else SCAN_ORDER
    outs_tab = FB_OUTS if fallback else OUTS
    pin_tab = [] if fallback else PIN_ORDER

    f16 = mybir.dt.float16
    nc = bass.Bass()
    gx_ext = nc.declare_dram_parameter(
        "gx", [LANES_PER_CORE, 2 * N], f16, isOutput=False
    )
    o_ext = nc.declare_dram_parameter("out", [LANES_PER_CORE, N], f16, isOutput=True)

    with OneWaitDrainTC(nc) as tc:
        with tc.tile_pool(name="p", bufs=1) as tp:
            gxt = [tp.tile([P, 2, N], f16, name=f"gx{t}") for t in range(LANE_TILES)]
            ot = [tp.tile([P, N], f16, name=f"o{t}") for t in range(LANE_TILES)]
            scr = tp.tile([P, 16], f16, name="scr")

            def gxd(t):
                return gx_ext[t * P : (t + 1) * P, :].rearrange(
                    "p (a n) -> p a n", n=N
                )

            engs = {"sp": nc.sync, "act": nc.scalar, "pool": nc.gpsimd}
            ring_last = {"sp": None, "act": None}
            in_dmas = {}  # (eng, tile, chunk) -> instruction

            def din(e, t, c):
                lo, hi = in_chunks[t][c], in_chunks[t][c + 1]
                d = engs[e].dma_start(
                    out=gxt[t][:, :, lo:hi], in_=gxd(t)[:, :, lo:hi]
                )
                if e in ring_last:
                    ov = CHAIN_OVERRIDE.get((e, t, c))
                    if ov is not None:
                        tile.add_dep_helper(
                            d.ins, in_dmas[ov].ins, sync=True, reason="lane chain"
                        )
                    elif ring_last[e] is not None:
                        tile.add_dep_helper(
                            d.ins, ring_last[e].ins, sync=True, reason="in chain"
                        )
                    ring_last[e] = d
                in_dmas[(e, t, c)] = d
                return d

            for e, t, c in ins_tab:
                din(e, t, c)

            nabs = [0]

            def scan(t, c):
                lo, hi = chunks[t][c], chunks[t][c + 1]
                if c > 0:
                    # absorber carries the input-DMA wait; the scan keeps
                    # only its carry-cell wait
                    nc.vector.tensor_copy(
                        scr[:, nabs[0] : nabs[0] + 1], gxt[t][:, 0, lo : lo + 1]
                    )
                    nabs[0] += 1
                init = 0.0 if c == 0 else ot[t][:, lo - 1 : lo]
                nc.vector.tensor_tensor_scan(
                    ot[t][:, lo:hi],
                    gxt[t][:, 0, lo:hi],
                    gxt[t][:, 1, lo:hi],
                    init,
                    mybir.AluOpType.mult,
                    mybir.AluOpType.add,
                )

            for t, c in scan_tab:
                scan(t, c)

            out_dmas = {}
            for e, t, lo, hi in outs_tab:
                d = engs[e].dma_start(
                    out=o_ext[t * P : (t + 1) * P, lo:hi], in_=ot[t][:, lo:hi]
                )
                out_dmas[(e, t, lo, hi)] = d

            # nosync pin chain: fixes HWDGE tick order (hence lane
            # round-robin) at compile time without any runtime waits
            prev = None
            for entry in pin_tab:
                if entry[0] == "in":
                    d = in_dmas[entry[1:]]
                else:
                    d = out_dmas[entry[1:]]
                if prev is not None:
                    tile.add_dep_helper(d.ins, prev.ins, sync=False, reason="pin")
                prev = d

    # one sync-wait per instruction is a hard walrus limit -- catch
    # regressions at build time rather than at NEFF compile
    for name, inst in nc.inst_map.items():
        si = inst.sync_info
        nw = len(si.on_wait) if si and si.on_wait else 0
        assert nw <= 1, f"{name} ({inst.engine}) carries {nw} sem waits"
    return nc


def _get_nc():
    global _NC_CACHE
    if _NC_CACHE is None:
        try:
            _NC_CACHE = _build_bass()
        except AssertionError:
            # one-wait audit failed -- fall back to the conservative
            # single-queue schedule rather than not running at all
            _NC_CACHE = _build_bass(fallback=True)
    return _NC_CACHE


def kernel(gates: np.ndarray, inputs: np.ndarray) -> np.ndarray:
    import os

    # The axon client here has no NTFF profile hook (antenv.axon_hooks);
    # make sure run_bass_kernel_spmd never takes the trace path even if
    # BASS_TRACE is set in the environment.
    os.environ["BASS_NEVER_TRACE"] = "1"
    from concourse.bass_utils import run_bass_kernel_spmd

    # [B, N, D] -> lane-major [B*D, N] fp16; pack gates|inputs along columns
    gt = np.asarray(gates, dtype=np.float32).transpose(0, 2, 1).reshape(LANES, N)
    xt = np.asarray(inputs, dtype=np.float32).transpose(0, 2, 1).reshape(LANES, N)
    gx = np.empty((LANES, 2 * N), dtype=np.float16)
    gx[:, :N] = gt
    gx[:, N:] = xt

    in_maps = [
        {"gx": gx[i * LANES_PER_CORE : (i + 1) * LANES_PER_CORE]}
        for i in range(N_CORES)
    ]
    try:
        res = run_bass_kernel_spmd(_get_nc(), in_maps, core_ids=list(range(N_CORES)))
    except Exception:
        # One retry: the device recovers from transient NRT execution
        # faults, and the NEFF is cached so the retry is cheap.
        res = run_bass_kernel_spmd(_get_nc(), in_maps, core_ids=list(range(N_CORES)))
    out = np.concatenate([res.results[i]["out"] for i in range(N_CORES)], axis=0)
    # [B*D, N] fp16 -> [B, N, D] f32
    return np.ascontiguousarray(
        out.astype(np.float32).reshape(B, D, N).transpose(0, 2, 1)
    )


# revision 27
# speedup vs baseline: 1.0207x; 1.0113x over previous
"""Trainium2 distributed kernel for a linear-recurrence associative scan.

    h_t = g_t * h_{t-1} + x_t  along the sequence axis (N=8192)

Shapes: gates/inputs [B=4, N=8192, D=1024] f32.

Strategy: the scan is independent per (b, d) lane -> 4096 lanes of length
8192.  Shard lanes across the 8 NeuronCores (512 lanes each), laid out
lane-major so each SBUF partition holds one lane's contiguous sequence and
the hardware scan instruction (tensor_tensor_scan: state = g*state + x along
the free dim, one recurrence per partition) does the whole recurrence at
line rate.  No collectives needed.

All HBM traffic is fp16 (the scan's recurrent state stays fp32 inside the
engine regardless of operand dtype, so the only error is per-element
quantization, ~1e-3 max rel err vs the 2e-2 gate).

The kernel is DVE-bound: the scan only runs on the vector engine (walrus
rejects TensorTensorScan on GpSimd), ~34us for 4 lane-tiles.  Each DMA's
transfer occupies only its issuing engine's queue, so the ~76us of HBM
traffic spreads over the three DMA-capable queues (SP, Activation,
Pool/GpSimd) at ~25us each, fully hidden under the scan chain.  Tiles
stream in chunks (small leading chunks so the first scan starts early,
small trailing chunks so the last output drains immediately).

Sync legality (walrus encodes at most ONE sem wait per instruction):
 * Every SBUF buffer is unique -- no slot-reuse WAR/WAW hazards.
 * Chained scan chunks would need two waits (input DMA + carry cell); a
   1-element copy absorber carries the DMA wait first, so the scan keeps
   only the carry wait.
 * HWDGE DMA completions share 8 global counting sems assigned round-robin
   in tick order; the 9th+ DMA carries a lane-reuse (WAW) wait.  Output
   DMAs also need a scan wait, so their lane wait must be elided: each
   HWDGE ring (SP / Act) chains its input DMAs with sync deps (~70ns each;
   the ring's engine clock then observes its own lanes) and issues at most
   3 trailing output DMAs, pinned (nosync deps) into a strict alternating
   global tick order so every out's lane predecessor is a chain-observed
   same-ring input.  Outputs spanning several scan chunks still carry ONE
   wait: all DVE ops tick one counting sem, so the join is a single value.
 * Pool's SWDGE DMAs use a separate 8-sem pool and stay under 8 total, so
   they never recycle and carry only their data wait.
 * Tile's kernel-tail drain would wait on every live sem at once; a custom
   drain splits it into a ladder of single-wait NOPs.
"""

import numpy as np

B, N, D = 4, 8192, 1024
N_CORES = 8
LANES = B * D  # 4096 independent recurrences
LANES_PER_CORE = LANES // N_CORES  # 512
P = 128  # SBUF partitions
LANE_TILES = LANES_PER_CORE // P  # 4

# per-tile scan-chunk boundaries (columns)
CHUNKS = {
    0: [0, 512, 1024, 4096, 8192],
    1: [0, 1024, 2048, 5120, 8192],
    2: [0, 2048, 5120, 8192],
    3: [0, 3072, 6144, 7168, 7680, 8192],
}
# per-tile input-DMA boundaries; every scan chunk must lie inside a single
# input DMA (so the scan carries at most one data wait)
IN_CHUNKS = {
    0: [0, 512, 1024, 4096, 8192],
    1: [0, 1024, 2048, 5120, 8192],
    2: [0, 2048, 5120, 8192],
    3: [0, 3072, 6144, 8192],
}
# input DMAs: (engine, tile, chunk) in emission order (= per-ring runtime
# order).  SP also carries tile 2's last chunk so Pool stays within the
# 8-sem SWDGE budget after taking two tile-3 output DMAs.
INS = [
    ("sp", 0, 0), ("act", 1, 0), ("pool", 2, 0),
    ("sp", 0, 1), ("act", 1, 1), ("pool", 2, 1),
    ("sp", 0, 2), ("act", 1, 2), ("pool", 3, 0),
    ("sp", 0, 3), ("act", 1, 3), ("pool", 3, 1),
    ("sp", 2, 2), ("pool", 3, 2),
]
# the 9th HWDGE input recycles lane L0; chaining it to its LANE PREDECESSOR
# makes the chain wait and the lane WAW wait one and the same sem wait
CHAIN_OVERRIDE = {("sp", 2, 2): ("sp", 0, 0)}
# DVE scan order: interleave tiles 0/1/2 by arrival, then tile 3
SCAN_ORDER = [
    (0, 0), (1, 0), (0, 1), (1, 1), (2, 0),
    (0, 2), (1, 2), (2, 1),
    (0, 3), (1, 3), (2, 2),
    (3, 0), (3, 1), (3, 2), (3, 3), (3, 4),
]
# outputs: (engine, tile, lo, hi); ring outs in readiness order, at most 3
# per HWDGE ring; tile 3's early chunks drain on the otherwise-idle Pool
OUTS = [
    ("sp", 0, 0, 8192),
    ("act", 1, 0, 8192),
    ("pool", 3, 0, 3072),
    ("sp", 2, 0, 8192),
    ("act", 3, 3072, 6144),
    ("pool", 3, 6144, 7168),
    ("pool", 3, 7168, 7680),
    ("act", 3, 7680, 8192),
]
# HWDGE compile-time tick order (fixes the 8-lane round-robin): ins first,
# arranged so each ring out's lane predecessor (8 ticks back) is an input
# on the SAME ring whose completion the ring's in-chain already observed.
# Labels refer to INS/OUTS entries.
PIN_ORDER = [
    ("in", "sp", 0, 0),   # L0
    ("in", "act", 1, 0),  # L1
    ("in", "sp", 0, 1),   # L2
    ("in", "act", 1, 1),  # L3
    ("in", "sp", 0, 2),   # L4
    ("in", "act", 1, 2),  # L5
    ("in", "sp", 0, 3),   # L6
    ("in", "act", 1, 3),  # L7
    ("in", "sp", 2, 2),   # L0 again; chain==lane wait via CHAIN_OVERRIDE
    ("out", "act", 1, 0, 8192),     # L1 <- act in(1,0), chain-observed
    ("out", "sp", 0, 0, 8192),      # L2 <- sp in(0,1), chain-observed
    ("out", "act", 3, 3072, 6144),  # L3 <- act in(1,1), chain-observed
    ("out", "sp", 2, 0, 8192),      # L4 <- sp in(0,2), chain-observed
    ("out", "act", 3, 7680, 8192),  # L5 <- act in(1,2), chain-observed
]

# Conservative fallback schedule (verified legal and correct on HW at
# ~78us): whole-tile chunks, everything on the SP queue -- exactly 8 HWDGE
# DMAs, no lane recycling, no pins needed.  Used only if the aggressive
# schedule ever fails the build-time one-wait audit (e.g. a Tile scheduler
# change shifts the lane rotation).
FB_CHUNKS = {t: [0, N] for t in range(LANE_TILES)}
FB_INS = [("sp", t, 0) for t in range(LANE_TILES)]
FB_SCAN_ORDER = [(t, 0) for t in range(LANE_TILES)]
FB_OUTS = [("sp", t, 0, N) for t in range(LANE_TILES)]

_NC_CACHE = None


def _build_bass(fallback=False):
    import concourse.bass as bass
    import concourse.tile as tile
    from concourse import mybir
    from concourse.vector_clock import ScopedClock, VectorClock

    class OneWaitDrainTC(tile.TileContext):
        """Split the kernel-tail drain's multi-sem wait into a ladder of
        single-wait NOPs (walrus allows one sync-wait per instruction)."""

        def _drain_and_barrier(self, tick_clock, wait_clock):
            full = tick_clock.global_clock
            n = len(full)
            for proc in range(n):
                t = full[proc]
                if t <= 0:
                    continue
                partial = VectorClock([0] * n)
                partial.require_at_least(proc, t)
                nop = self.nc.sync.nop(hint=f"drainwait{proc}")
                wait_clock.add_sem_waits(nop.ins, ScopedClock({None: partial}))
            self.nc.sync.drain()
            self.nc.all_engine_barrier()
            assert self.sems is not None
            popped = self.nc._tile_sem_poison_stack.pop()
            assert popped is self._sem_poison
            self.nc.clear_and_free_semaphores(list(self.sems.allocated().values()))

    chunks = FB_CHUNKS if fallback else CHUNKS
    in_chunks = FB_CHUNKS if fallback else IN_CHUNKS
    ins_tab = FB_INS if fallback else INS
    scan_tab = FB_SCAN_ORDER if fallback else SCAN_ORDER
    outs_tab = FB_OUTS if fallback else OUTS
    pin_tab = [] if fallback else PIN_ORDER

    f16 = mybir.dt.float16
    nc = bass.Bass()
    gx_ext = nc.declare_dram_parameter(
        "gx", [LANES_PER_CORE, 2 * N], f16, isOutput=False
    )
    o_ext = nc.declare_dram_parameter("out", [LANES_PER_CORE, N], f16, isOutput=True)

    with OneWaitDrainTC(nc) as tc:
        with tc.tile_pool(name="p", bufs=1) as tp:
            gxt = [tp.tile([P, 2, N], f16, name=f"gx{t}") for t in range(LANE_TILES)]
            ot = [tp.tile([P, N], f16, name=f"o{t}") for t in range(LANE_TILES)]
            scr = tp.tile([P, 16], f16, name="scr")

            def gxd(t):
                return gx_ext[t * P : (t + 1) * P, :].rearrange(
                    "p (a n) -> p a n", n=N
                )

            engs = {"sp": nc.sync, "act": nc.scalar, "pool": nc.gpsimd}
            ring_last = {"sp": None, "act": None}
            in_dmas = {}  # (eng, tile, chunk) -> instruction

            def din(e, t, c):
                lo, hi = in_chunks[t][c], in_chunks[t][c + 1]
                d = engs[e].dma_start(
                    out=gxt[t][:, :, lo:hi], in_=gxd(t)[:, :, lo:hi]
                )
                if e in ring_last:
                    ov = CHAIN_OVERRIDE.get((e, t, c))
                    if ov is not None:
                        tile.add_dep_helper(
                            d.ins, in_dmas[ov].ins, sync=True, reason="lane chain"
                        )
                    elif ring_last[e] is not None:
                        tile.add_dep_helper(
                            d.ins, ring_last[e].ins, sync=True, reason="in chain"
                        )
                    ring_last[e] = d
                in_dmas[(e, t, c)] = d
                return d

            for e, t, c in ins_tab:
                din(e, t, c)

            nabs = [0]

            def scan(t, c):
                lo, hi = chunks[t][c], chunks[t][c + 1]
                if c > 0:
                    # absorber carries the input-DMA wait; the scan keeps
                    # only its carry-cell wait
                    nc.vector.tensor_copy(
                        scr[:, nabs[0] : nabs[0] + 1], gxt[t][:, 0, lo : lo + 1]
                    )
                    nabs[0] += 1
                init = 0.0 if c == 0 else ot[t][:, lo - 1 : lo]
                nc.vector.tensor_tensor_scan(
                    ot[t][:, lo:hi],
                    gxt[t][:, 0, lo:hi],
                    gxt[t][:, 1, lo:hi],
                    init,
                    mybir.AluOpType.mult,
                    mybir.AluOpType.add,
                )

            for t, c in scan_tab:
                scan(t, c)

            out_dmas = {}
            for e, t, lo, hi in outs_tab:
                d = engs[e].dma_start(
                    out=o_ext[t * P : (t + 1) * P, lo:hi], in_=ot[t][:, lo:hi]
                )
                out_dmas[(e, t, lo, hi)] = d

            # nosync pin chain: fixes HWDGE tick order (hence lane
            # round-robin) at compile time without any runtime waits
            prev = None
            for entry in pin_tab:
                if entry[0] == "in":
                    d = in_dmas[entry[1:]]
                else:
                    d = out_dmas[entry[1:]]
                if prev is not None:
                    tile.add_dep_helper(d.ins, prev.ins, sync=False, reason="pin")
                prev = d

    # one sync-wait per instruction is a hard walrus limit -- catch
    # regressions at build time rather than at NEFF compile
    for name, inst in nc.inst_map.items():
        si = inst.sync_info
        nw = len(si.on_wait) if si and si.on_wait else 0
        assert nw <= 1, f"{name} ({inst.engine}) carries {nw} sem waits"
    return nc


def _get_nc():
    global _NC_CACHE
    if _NC_CACHE is None:
        try:
            _NC_CACHE = _build_bass()
        except AssertionError:
            # one-wait audit failed -- fall back to the conservative
            # single-queue schedule rather than not running at all
            _NC_CACHE = _build_bass(fallback=True)
    return _NC_CACHE


def kernel(gates: np.ndarray, inputs: np.ndarray) -> np.ndarray:
    import os

    # The axon client here has no NTFF profile hook (antenv.axon_hooks);
    # make sure run_bass_kernel_spmd never takes the trace path even if
    # BASS_TRACE is set in the environment.
    os.environ["BASS_NEVER_TRACE"] = "1"
    from concourse.bass_utils import run_bass_kernel_spmd

    # [B, N, D] -> lane-major [B*D, N] fp16; pack gates|inputs along columns
    gt = np.asarray(gates, dtype=np.float32).transpose(0, 2, 1).reshape(LANES, N)
    xt = np.asarray(inputs, dtype=np.float32).transpose(0, 2, 1).reshape(LANES, N)
    gx = np.empty((LANES, 2 * N), dtype=np.float16)
    gx[:, :N] = gt
    gx[:, N:] = xt

    in_maps = [
        {"gx": gx[i * LANES_PER_CORE : (i + 1) * LANES_PER_CORE]}
        for i in range(N_CORES)
    ]
    try:
        res = run_bass_kernel_spmd(_get_nc(), in_maps, core_ids=list(range(N_CORES)))
    except Exception:
        # One retry: the device recovers from transient NRT execution
        # faults, and the NEFF is cached so the retry is cheap.
        res = run_bass_kernel_spmd(_get_nc(), in_maps, core_ids=list(range(N_CORES)))
    out = np.concatenate([res.results[i]["out"] for i in range(N_CORES)], axis=0)
    # [B*D, N] fp16 -> [B, N, D] f32
    return np.ascontiguousarray(
        out.astype(np.float32).reshape(B, D, N).transpose(0, 2, 1)
    )


# revision 28
# speedup vs baseline: 1.0220x; 1.0013x over previous
"""Trainium2 distributed kernel for a linear-recurrence associative scan.

    h_t = g_t * h_{t-1} + x_t  along the sequence axis (N=8192)

Shapes: gates/inputs [B=4, N=8192, D=1024] f32.

Strategy: the scan is independent per (b, d) lane -> 4096 lanes of length
8192.  Shard lanes across the 8 NeuronCores (512 lanes each), laid out
lane-major so each SBUF partition holds one lane's contiguous sequence and
the hardware scan instruction (tensor_tensor_scan: state = g*state + x along
the free dim, one recurrence per partition) does the whole recurrence at
line rate.  No collectives needed.

All HBM traffic is fp16 (the scan's recurrent state stays fp32 inside the
engine regardless of operand dtype, so the only error is per-element
quantization, ~1e-3 max rel err vs the 2e-2 gate).

The kernel is DVE-bound: the scan only runs on the vector engine (walrus
rejects TensorTensorScan on GpSimd), ~34us for 4 lane-tiles.  Each DMA's
transfer occupies only its issuing engine's queue, so the ~76us of HBM
traffic spreads over the three DMA-capable queues (SP, Activation,
Pool/GpSimd) at ~25us each, fully hidden under the scan chain.  Tiles
stream in chunks (small leading chunks so the first scan starts early,
small trailing chunks so the last output drains immediately).

Sync legality (walrus encodes at most ONE sem wait per instruction):
 * Every SBUF buffer is unique -- no slot-reuse WAR/WAW hazards.
 * Chained scan chunks would need two waits (input DMA + carry cell); a
   1-element copy absorber carries the DMA wait first, so the scan keeps
   only the carry wait.
 * HWDGE DMA completions share 8 global counting sems assigned round-robin
   in tick order; the 9th+ DMA carries a lane-reuse (WAW) wait.  Output
   DMAs also need a scan wait, so their lane wait must be elided: each
   HWDGE ring (SP / Act) chains its input DMAs with sync deps (~70ns each;
   the ring's engine clock then observes its own lanes) and issues at most
   3 trailing output DMAs, pinned (nosync deps) into a strict alternating
   global tick order so every out's lane predecessor is a chain-observed
   same-ring input.  Outputs spanning several scan chunks still carry ONE
   wait: all DVE ops tick one counting sem, so the join is a single value.
 * Pool's SWDGE DMAs use a separate 8-sem pool and stay under 8 total, so
   they never recycle and carry only their data wait.
 * Tile's kernel-tail drain would wait on every live sem at once; a custom
   drain splits it into a ladder of single-wait NOPs.
"""

import numpy as np

B, N, D = 4, 8192, 1024
N_CORES = 8
LANES = B * D  # 4096 independent recurrences
LANES_PER_CORE = LANES // N_CORES  # 512
P = 128  # SBUF partitions
LANE_TILES = LANES_PER_CORE // P  # 4

# per-tile scan-chunk boundaries (columns)
CHUNKS = {
    0: [0, 512, 1024, 4096, 8192],
    1: [0, 1024, 2048, 5120, 8192],
    2: [0, 2048, 5120, 8192],
    3: [0, 3072, 6144, 7168, 7680, 8192],
}
# per-tile input-DMA boundaries; every scan chunk must lie inside a single
# input DMA (so the scan carries at most one data wait)
IN_CHUNKS = {
    0: [0, 512, 1024, 4096, 8192],
    1: [0, 1024, 2048, 5120, 8192],
    2: [0, 2048, 8192],
    3: [0, 3072, 6144, 8192],
}
# input DMAs: (engine, tile, chunk) in emission order (= per-ring runtime
# order).  SP also carries tile 2's last chunk so Pool stays within the
# 8-sem SWDGE budget after taking two tile-3 output DMAs.
INS = [
    ("sp", 0, 0), ("act", 1, 0), ("pool", 2, 0),
    ("sp", 0, 1), ("act", 1, 1), ("pool", 2, 1),
    ("sp", 0, 2), ("act", 1, 2), ("pool", 3, 0),
    ("sp", 0, 3), ("act", 1, 3), ("pool", 3, 1),
    ("pool", 3, 2),
]
CHAIN_OVERRIDE = {}
# DVE scan order: interleave tiles 0/1/2 by arrival, then tile 3
SCAN_ORDER = [
    (0, 0), (1, 0), (0, 1), (1, 1), (2, 0),
    (0, 2), (1, 2), (2, 1),
    (0, 3), (1, 3), (2, 2),
    (3, 0), (3, 1), (3, 2), (3, 3), (3, 4),
]
# outputs: (engine, tile, lo, hi); ring outs in readiness order, at most 3
# per HWDGE ring; tile 3's early chunks drain on the otherwise-idle Pool
OUTS = [
    ("sp", 0, 0, 8192),
    ("act", 1, 0, 8192),
    ("act", 3, 0, 3072),
    ("sp", 2, 0, 8192),
    ("sp", 3, 3072, 6144),
    ("pool", 3, 6144, 7168),
    ("pool", 3, 7168, 7680),
    ("act", 3, 7680, 8192),
]
# HWDGE compile-time tick order (fixes the 8-lane round-robin): ins first,
# arranged so each ring out's lane predecessor (8 ticks back) is an input
# on the SAME ring whose completion the ring's in-chain already observed.
# Labels refer to INS/OUTS entries.
PIN_ORDER = [
    ("in", "sp", 0, 0),   # L0
    ("in", "act", 1, 0),  # L1
    ("in", "sp", 0, 1),   # L2
    ("in", "act", 1, 1),  # L3
    ("in", "sp", 0, 2),   # L4
    ("in", "act", 1, 2),  # L5
    ("in", "sp", 0, 3),   # L6
    ("in", "act", 1, 3),  # L7
    ("out", "sp", 0, 0, 8192),      # L0 <- sp in(0,0), chain-observed
    ("out", "act", 1, 0, 8192),     # L1 <- act in(1,0), chain-observed
    ("out", "sp", 2, 0, 8192),      # L2 <- sp in(0,1), chain-observed
    ("out", "act", 3, 0, 3072),     # L3 <- act in(1,1), chain-observed
    ("out", "sp", 3, 3072, 6144),   # L4 <- sp in(0,2), chain-observed
    ("out", "act", 3, 7680, 8192),  # L5 <- act in(1,2), chain-observed
]

# Conservative fallback schedule (verified legal and correct on HW at
# ~78us): whole-tile chunks, everything on the SP queue -- exactly 8 HWDGE
# DMAs, no lane recycling, no pins needed.  Used only if the aggressive
# schedule ever fails the build-time one-wait audit (e.g. a Tile scheduler
# change shifts the lane rotation).
FB_CHUNKS = {t: [0, N] for t in range(LANE_TILES)}
FB_INS = [("sp", t, 0) for t in range(LANE_TILES)]
FB_SCAN_ORDER = [(t, 0) for t in range(LANE_TILES)]
FB_OUTS = [("sp", t, 0, N) for t in range(LANE_TILES)]

_NC_CACHE = None


def _build_bass(fallback=False):
    import concourse.bass as bass
    import concourse.tile as tile
    from concourse import mybir
    from concourse.vector_clock import ScopedClock, VectorClock

    class OneWaitDrainTC(tile.TileContext):
        """Split the kernel-tail drain's multi-sem wait into a ladder of
        single-wait NOPs (walrus allows one sync-wait per instruction)."""

        def _drain_and_barrier(self, tick_clock, wait_clock):
            full = tick_clock.global_clock
            n = len(full)
            for proc in range(n):
                t = full[proc]
                if t <= 0:
                    continue
                partial = VectorClock([0] * n)
                partial.require_at_least(proc, t)
                nop = self.nc.sync.nop(hint=f"drainwait{proc}")
                wait_clock.add_sem_waits(nop.ins, ScopedClock({None: partial}))
            self.nc.sync.drain()
            self.nc.all_engine_barrier()
            assert self.sems is not None
            popped = self.nc._tile_sem_poison_stack.pop()
            assert popped is self._sem_poison
            self.nc.clear_and_free_semaphores(list(self.sems.allocated().values()))

    chunks = FB_CHUNKS if fallback else CHUNKS
    in_chunks = FB_CHUNKS if fallback else IN_CHUNKS
    ins_tab = FB_INS if fallback else INS
    scan_tab = FB_SCAN_ORDER if fallback else SCAN_ORDER
    outs_tab = FB_OUTS if fallback else OUTS
    pin_tab = [] if fallback else PIN_ORDER

    f16 = mybir.dt.float16
    nc = bass.Bass()
    gx_ext = nc.declare_dram_parameter(
        "gx", [LANES_PER_CORE, 2 * N], f16, isOutput=False
    )
    o_ext = nc.declare_dram_parameter("out", [LANES_PER_CORE, N], f16, isOutput=True)

    with OneWaitDrainTC(nc) as tc:
        with tc.tile_pool(name="p", bufs=1) as tp:
            gxt = [tp.tile([P, 2, N], f16, name=f"gx{t}") for t in range(LANE_TILES)]
            ot = [tp.tile([P, N], f16, name=f"o{t}") for t in range(LANE_TILES)]
            scr = tp.tile([P, 16], f16, name="scr")

            def gxd(t):
                return gx_ext[t * P : (t + 1) * P, :].rearrange(
                    "p (a n) -> p a n", n=N
                )

            engs = {"sp": nc.sync, "act": nc.scalar, "pool": nc.gpsimd}
            ring_last = {"sp": None, "act": None}
            in_dmas = {}  # (eng, tile, chunk) -> instruction

            def din(e, t, c):
                lo, hi = in_chunks[t][c], in_chunks[t][c + 1]
                d = engs[e].dma_start(
                    out=gxt[t][:, :, lo:hi], in_=gxd(t)[:, :, lo:hi]
                )
                if e in ring_last:
                    ov = CHAIN_OVERRIDE.get((e, t, c))
                    if ov is not None:
                        tile.add_dep_helper(
                            d.ins, in_dmas[ov].ins, sync=True, reason="lane chain"
                        )
                    elif ring_last[e] is not None:
                        tile.add_dep_helper(
                            d.ins, ring_last[e].ins, sync=True, reason="in chain"
                        )
                    ring_last[e] = d
                in_dmas[(e, t, c)] = d
                return d

            for e, t, c in ins_tab:
                din(e, t, c)

            nabs = [0]

            def scan(t, c):
                lo, hi = chunks[t][c], chunks[t][c + 1]
                if c > 0:
                    # absorber carries the input-DMA wait; the scan keeps
                    # only its carry-cell wait
                    nc.vector.tensor_copy(
                        scr[:, nabs[0] : nabs[0] + 1], gxt[t][:, 0, lo : lo + 1]
                    )
                    nabs[0] += 1
                init = 0.0 if c == 0 else ot[t][:, lo - 1 : lo]
                nc.vector.tensor_tensor_scan(
                    ot[t][:, lo:hi],
                    gxt[t][:, 0, lo:hi],
                    gxt[t][:, 1, lo:hi],
                    init,
                    mybir.AluOpType.mult,
                    mybir.AluOpType.add,
                )

            for t, c in scan_tab:
                scan(t, c)

            out_dmas = {}
            for e, t, lo, hi in outs_tab:
                d = engs[e].dma_start(
                    out=o_ext[t * P : (t + 1) * P, lo:hi], in_=ot[t][:, lo:hi]
                )
                out_dmas[(e, t, lo, hi)] = d

            # nosync pin chain: fixes HWDGE tick order (hence lane
            # round-robin) at compile time without any runtime waits
            prev = None
            for entry in pin_tab:
                if entry[0] == "in":
                    d = in_dmas[entry[1:]]
                else:
                    d = out_dmas[entry[1:]]
                if prev is not None:
                    tile.add_dep_helper(d.ins, prev.ins, sync=False, reason="pin")
                prev = d

    # one sync-wait per instruction is a hard walrus limit -- catch
    # regressions at build time rather than at NEFF compile
    for name, inst in nc.inst_map.items():
        si = inst.sync_info
        nw = len(si.on_wait) if si and si.on_wait else 0
        assert nw <= 1, f"{name} ({inst.engine}) carries {nw} sem waits"
    return nc


def _get_nc():
    global _NC_CACHE
    if _NC_CACHE is None:
        try:
            _NC_CACHE = _build_bass()
        except AssertionError:
            # one-wait audit failed -- fall back to the conservative
            # single-queue schedule rather than not running at all
            _NC_CACHE = _build_bass(fallback=True)
    return _NC_CACHE


def kernel(gates: np.ndarray, inputs: np.ndarray) -> np.ndarray:
    import os

    # The axon client here has no NTFF profile hook (antenv.axon_hooks);
    # make sure run_bass_kernel_spmd never takes the trace path even if
    # BASS_TRACE is set in the environment.
    os.environ["BASS_NEVER_TRACE"] = "1"
    from concourse.bass_utils import run_bass_kernel_spmd

    # [B, N, D] -> lane-major [B*D, N] fp16; pack gates|inputs along columns
    gt = np.asarray(gates, dtype=np.float32).transpose(0, 2, 1).reshape(LANES, N)
    xt = np.asarray(inputs, dtype=np.float32).transpose(0, 2, 1).reshape(LANES, N)
    gx = np.empty((LANES, 2 * N), dtype=np.float16)
    gx[:, :N] = gt
    gx[:, N:] = xt

    in_maps = [
        {"gx": gx[i * LANES_PER_CORE : (i + 1) * LANES_PER_CORE]}
        for i in range(N_CORES)
    ]
    try:
        res = run_bass_kernel_spmd(_get_nc(), in_maps, core_ids=list(range(N_CORES)))
    except Exception:
        # One retry: the device recovers from transient NRT execution
        # faults, and the NEFF is cached so the retry is cheap.
        res = run_bass_kernel_spmd(_get_nc(), in_maps, core_ids=list(range(N_CORES)))
    out = np.concatenate([res.results[i]["out"] for i in range(N_CORES)], axis=0)
    # [B*D, N] fp16 -> [B, N, D] f32
    return np.ascontiguousarray(
        out.astype(np.float32).reshape(B, D, N).transpose(0, 2, 1)
    )
